# revision 8
# baseline (speedup 1.0000x reference)
"""MiniMax-M2 sparse MoE block on 8 Trainium2 NeuronCores.

Strategy: expert-parallel. Core c owns expert c's weights. The host computes
the routing (top-2 of 8, sigmoid scores + bias for selection) in float64,
gathers each expert's tokens, and ships them pre-transposed; each core runs
the gated FFN (silu(x@w1.T) * (x@w3.T)) @ w2.T over its gathered tokens in
float32r (full-rate fp32 matmul mode) and also computes the router logits for
its 1/8 slice of tokens. The host scatter-adds the weighted expert outputs.

Shapes (fixed by the problem): B=4, S=4096, H=2048, I=1024, E=8, top_k=2.

The intermediate dim I is processed in two halves so that one half's weights
(w1/w3 halves [H,512] + w2 half [512,H], fp32) stay resident in SBUF; each
half emits a partial Y (summed on the host). The router block is emitted
between the two halves so its matmuls run on a warm PE and overlap the
half-1 weight reload.
"""
import numpy as np

import concourse.bass as bass
import concourse.mybir as mybir
from concourse import bacc
import concourse.tile as tile
from concourse.bass_utils import run_bass_kernel_spmd

H = 2048
I = 1024
E = 8
TOP_K = 2
KT_H = H // 128          # 16 k-tiles over H
IH = I // 2              # 512: half of intermediate dim
ISUB = IH // 128         # 4 psum subtiles per half
KT_I = IH // 128         # 4 k-tiles over I-half
NTOK = 512               # full token tile (matmul moving dim)
TAIL = 256               # token padding granularity (f32r full rate needs >=256)
f32 = mybir.dt.float32
f32r = mybir.dt.float32r
ACT_FN = mybir.ActivationFunctionType.Silu  # test_sim overrides (CoreSim lacks Silu)


def _ensure_ntff_hook():
    """If the environment requests tracing (BASS_TRACE) but lacks the
    antenv.axon_hooks module, inject it so profiling works instead of being
    silently skipped."""
    try:
        from antenv.axon_hooks import get_axon_ntff_profile_hook  # noqa: F401
        return
    except ImportError:
        pass
    try:
        import sys, types
        import antenv
        from trn_agent_boot.trn_boot import _ntff_profile_via_ctypes
        mod = types.ModuleType("antenv.axon_hooks")
        mod._hook = _ntff_profile_via_ctypes("/opt/axon/libaxon_pjrt.so")
        mod.set_axon_ntff_profile_hook = lambda h: setattr(mod, "_hook", h)
        mod.get_axon_ntff_profile_hook = lambda: mod._hook
        sys.modules["antenv.axon_hooks"] = mod
        antenv.axon_hooks = mod
    except Exception:
        pass


_BUILD_CACHE = {}
LAST_RESULT = None


def build(r_tokens, tslice):
    """Build the SPMD program: R gathered tokens per core (multiple of TAIL)
    and a router slice of `tslice` tokens per core."""
    key = (r_tokens, tslice)
    if key in _BUILD_CACHE:
        return _BUILD_CACHE[key]
    assert r_tokens % TAIL == 0 and tslice % NTOK == 0
    ns_r = tslice // NTOK
    widths = [NTOK] * (r_tokens // NTOK)
    if r_tokens % NTOK:
        widths.append(TAIL)
    elif len(widths) >= 2:
        widths = widths[:-1] + [TAIL, TAIL]  # small last tiles: cheaper drain
    offs = np.cumsum([0] + widths)[:-1].tolist()
    R = r_tokens

    nc = bacc.Bacc("TRN2", target_bir_lowering=False, debug=False, num_devices=8)
    xtb = nc.dram_tensor("xtb", [H, R], f32r, kind="ExternalInput").ap()
    cwb = nc.dram_tensor("cwb", [128, R], f32, kind="ExternalInput").ap()
    w1tb = nc.dram_tensor("w1tb", [2, KT_H, 128, IH], f32r, kind="ExternalInput").ap()
    w3tb = nc.dram_tensor("w3tb", [2, KT_H, 128, IH], f32r, kind="ExternalInput").ap()
    w2tb = nc.dram_tensor("w2tb", [2, KT_I, 128, H], f32r, kind="ExternalInput").ap()
    xrb = nc.dram_tensor("xrb", [ns_r, KT_H, 128, NTOK], f32r, kind="ExternalInput").ap()
    gwtb = nc.dram_tensor("gwtb", [KT_H, 128, E], f32r, kind="ExternalInput").ap()
    yb = nc.dram_tensor("yb", [2, H, R], f32, kind="ExternalOutput").ap()
    logb = nc.dram_tensor("logb", [E, tslice], f32, kind="ExternalOutput").ap()

    xtb_k = xtb.rearrange("(kt p) r -> kt p r", p=128)

    with tile.TileContext(nc) as tc:
        with (
            tc.tile_pool(name="wp", bufs=1) as wp,
            tc.tile_pool(name="xp", bufs=2) as xp,
            tc.tile_pool(name="pp", bufs=2) as pp,
            tc.tile_pool(name="cp", bufs=2) as cp,
            tc.tile_pool(name="yp", bufs=4) as yp,
            tc.tile_pool(name="rp", bufs=2) as rp,
            tc.tile_pool(name="gups", bufs=4, space="PSUM") as gups,
            tc.tile_pool(name="yps", bufs=3, space="PSUM") as yps,
            tc.tile_pool(name="lps", bufs=1, space="PSUM") as lps,
        ):
            def ffn_half(h):
                w1t, w3t, w2t = [], [], []
                for ki in range(KT_H):
                    t1 = wp.tile([128, IH], f32r, tag=f"w1_{ki}", name=f"w1_{h}_{ki}")
                    nc.sync.dma_start(out=t1[:], in_=w1tb[h, ki])
                    w1t.append(t1)
                    t3 = wp.tile([128, IH], f32r, tag=f"w3_{ki}", name=f"w3_{h}_{ki}")
                    nc.sync.dma_start(out=t3[:], in_=w3tb[h, ki])
                    w3t.append(t3)
                for ki in range(KT_I):
                    t2 = wp.tile([128, H], f32r, tag=f"w2_{ki}", name=f"w2_{h}_{ki}")
                    nc.sync.dma_start(out=t2[:], in_=w2tb[h, ki])
                    w2t.append(t2)

                for ti, (off, w) in enumerate(zip(offs, widths)):
                    if h == 0 and ti < len(router_slices):
                        router_slices[ti]()
                    xt = xp.tile([128, KT_H * w], f32r, tag="x", name=f"x_{h}_{ti}")
                    for ki in range(KT_H):
                        nc.sync.dma_start(
                            out=xt[:, ki * w:(ki + 1) * w],
                            in_=xtb_k[ki][:, off:off + w],
                        )
                    cw = cp.tile([128, w], f32, tag="cw", name=f"cw_{h}_{ti}")
                    nc.sync.dma_start(out=cw[:], in_=cwb[:, off:off + w])

                    pt = pp.tile([128, ISUB * w], f32r, tag="p", name=f"p_{h}_{ti}")
                    for isub in range(ISUB):
                        gp = gups.tile([128, w], f32, tag="gu", name=f"g_{h}_{ti}_{isub}")
                        up = gups.tile([128, w], f32, tag="gu", name=f"u_{h}_{ti}_{isub}")
                        msl = slice(isub * 128, (isub + 1) * 128)
                        for ki in range(KT_H):
                            nc.tensor.matmul(
                                gp[:],
                                w1t[ki][:, msl],
                                xt[:, ki * w:(ki + 1) * w],
                                start=(ki == 0),
                                stop=(ki == KT_H - 1),
                            )
                        for ki in range(KT_H):
                            nc.tensor.matmul(
                                up[:],
                                w3t[ki][:, msl],
                                xt[:, ki * w:(ki + 1) * w],
                                start=(ki == 0),
                                stop=(ki == KT_H - 1),
                            )
                        psl = slice(isub * w, (isub + 1) * w)
                        sg = yp.tile([128, w], f32, tag="sg", bufs=2, name=f"sg_{h}_{ti}_{isub}")
                        nc.scalar.activation(sg[:], gp[:], ACT_FN)
                        # P = silu(G) * (U * cw); cw folded here (per-token scale)
                        nc.vector.tensor_mul(pt[:, psl], up[:], cw[:])
                        nc.vector.tensor_mul(pt[:, psl], pt[:, psl], sg[:])
                    for m in range(KT_H):
                        yps_t = yps.tile([128, w], f32, tag="y", name=f"y_{h}_{ti}_{m}")
                        for ki in range(KT_I):
                            nc.tensor.matmul(
                                yps_t[:],
                                w2t[ki][:, m * 128:(m + 1) * 128],
                                pt[:, ki * w:(ki + 1) * w],
                                start=(ki == 0),
                                stop=(ki == KT_I - 1),
                            )
                        ysb = yp.tile([128, w], f32, tag="ysb", bufs=3, name=f"ysb_{h}_{ti}_{m}")
                        nc.vector.tensor_copy(ysb[:], yps_t[:])
                        nc.sync.dma_start(
                            out=yb[h][m * 128:(m + 1) * 128, off:off + w], in_=ysb[:]
                        )

            gwt_box = []

            def router_slice(s):
                def emit():
                    if not gwt_box:
                        g = rp.tile([128, KT_H * E], f32r, tag="gwt", bufs=1, name="gwt")
                        for ki in range(KT_H):
                            nc.sync.dma_start(
                                out=g[:, ki * E:(ki + 1) * E], in_=gwtb[ki]
                            )
                        gwt_box.append(g)
                    gwt = gwt_box[0]
                    xr = xp.tile([128, KT_H * NTOK], f32r, tag="x", name=f"xr_{s}")
                    for ki in range(KT_H):
                        nc.sync.dma_start(
                            out=xr[:, ki * NTOK:(ki + 1) * NTOK], in_=xrb[s, ki]
                        )
                    lp = lps.tile([E, NTOK], f32, tag="lp", name=f"lp_{s}")
                    for ki in range(KT_H):
                        nc.tensor.matmul(
                            lp[:],
                            gwt[:, ki * E:(ki + 1) * E],
                            xr[:, ki * NTOK:(ki + 1) * NTOK],
                            start=(ki == 0),
                            stop=(ki == KT_H - 1),
                        )
                    lsb = rp.tile([E, NTOK], f32, tag="lsb", name=f"lsb_{s}")
                    nc.vector.tensor_copy(lsb[:], lp[:])
                    nc.sync.dma_start(out=logb[:, s * NTOK:(s + 1) * NTOK], in_=lsb[:])
                return emit

            router_slices = [router_slice(s) for s in range(ns_r)]
            ffn_half(0)
            ffn_half(1)

    nc.compile()
    _BUILD_CACHE[key] = nc
    return nc


def kernel(hidden_states, gate_w, e_bias, w1, w2, w3):
    _ensure_ntff_hook()
    B, S, Hd = hidden_states.shape
    assert Hd == H
    x = np.ascontiguousarray(hidden_states.reshape(-1, H).astype(np.float32))
    T = x.shape[0]
    tslice = T // 8

    # ---- host routing (float64 for stable top-k decisions) ----
    logits64 = x.astype(np.float64) @ gate_w.T.astype(np.float64)
    scores = 1.0 / (1.0 + np.exp(-logits64))
    biased = scores + e_bias.astype(np.float64)
    sel = np.argsort(-biased, axis=1, kind="stable")[:, :TOP_K]       # [T, 2]
    rw = np.take_along_axis(scores, sel, axis=1)                       # [T, 2]
    rw = rw / np.maximum(rw.sum(-1, keepdims=True), 1e-12)
    rw = rw.astype(np.float32)

    idx = []
    wts = []
    for e in range(E):
        mask = sel == e                                                # [T, 2]
        ide = np.where(mask.any(1))[0]
        pos = mask[ide].argmax(1)
        idx.append(ide)
        wts.append(rw[ide, pos])

    nmax = max(1, max(len(i) for i in idx))
    R = ((nmax + TAIL - 1) // TAIL) * TAIL

    nc = build(R, tslice)

    # ---- per-core inputs ----
    w1T = np.ascontiguousarray(np.transpose(w1, (0, 2, 1)).astype(np.float32))  # [E, H, I]
    w3T = np.ascontiguousarray(np.transpose(w3, (0, 2, 1)).astype(np.float32))  # [E, H, I]
    w2T = np.ascontiguousarray(np.transpose(w2, (0, 2, 1)).astype(np.float32))  # [E, I, H]
    gwtb = np.ascontiguousarray(gate_w.T.astype(np.float32).reshape(KT_H, 128, E))

    in_maps = []
    for c in range(E):
        n_c = len(idx[c])
        xg = np.zeros((R, H), np.float32)
        xg[:n_c] = x[idx[c]]
        xtb = np.ascontiguousarray(xg.T)                               # [H, R]
        cwf = np.zeros((R,), np.float32)
        cwf[:n_c] = wts[c]
        cwb = np.ascontiguousarray(np.broadcast_to(cwf, (128, R)))
        w1tb = np.ascontiguousarray(
            w1T[c].reshape(KT_H, 128, 2, IH).transpose(2, 0, 1, 3)
        )  # [2, KT_H, 128, IH]
        w3tb = np.ascontiguousarray(
            w3T[c].reshape(KT_H, 128, 2, IH).transpose(2, 0, 1, 3)
        )
        w2tb = np.ascontiguousarray(w2T[c].reshape(2, KT_I, 128, H))
        xsl = x[c * tslice:(c + 1) * tslice]                           # [tslice, H]
        xrb = np.ascontiguousarray(
            xsl.reshape(tslice // NTOK, NTOK, H).transpose(0, 2, 1)
            .reshape(tslice // NTOK, KT_H, 128, NTOK)
        )
        in_maps.append(
            {
                "xtb": xtb,
                "cwb": cwb,
                "w1tb": w1tb,
                "w3tb": w3tb,
                "w2tb": w2tb,
                "xrb": xrb,
                "gwtb": gwtb,
            }
        )

    res = run_bass_kernel_spmd(nc, in_maps, list(range(8)))
    global LAST_RESULT
    LAST_RESULT = res

    # ---- combine on host ----
    out = np.zeros((T, H), np.float32)
    logits_out = np.empty((T, E), np.float32)
    for c in range(E):
        r = res.results[c]
        y = (r["yb"][0] + r["yb"][1]).T                                # [R, H]
        out[idx[c]] += y[: len(idx[c])]
        logits_out[c * tslice:(c + 1) * tslice] = r["logb"].T
    return out.reshape(B, S, H), logits_out


# revision 11
# speedup vs baseline: 1.0407x; 1.0407x over previous
"""MiniMax-M2 sparse MoE block on 8 Trainium2 NeuronCores.

Strategy: expert-parallel. Core c owns expert c's weights. The host computes
the routing (top-2 of 8, sigmoid scores + bias for selection) in float64,
gathers each expert's tokens, and ships them pre-transposed; each core runs
the gated FFN (silu(x@w1.T) * (x@w3.T)) @ w2.T over its gathered tokens in
float32r (full-rate fp32 matmul mode) and also computes the router logits for
its 1/8 slice of tokens. The host scatter-adds the weighted expert outputs.

Shapes (fixed by the problem): B=4, S=4096, H=2048, I=1024, E=8, top_k=2.

The intermediate dim I is processed in two halves so that one half's weights
(w1/w3 halves [H,512] + w2 half [512,H], fp32) stay resident in SBUF; each
half emits a partial Y (summed on the host). The router block is emitted
between the two halves so its matmuls run on a warm PE and overlap the
half-1 weight reload.
"""
import numpy as np

import concourse.bass as bass
import concourse.mybir as mybir
from concourse import bacc
import concourse.tile as tile
from concourse.bass_utils import run_bass_kernel_spmd

H = 2048
I = 1024
E = 8
TOP_K = 2
KT_H = H // 128          # 16 k-tiles over H
IH = I // 2              # 512: half of intermediate dim
ISUB = IH // 128         # 4 psum subtiles per half
KT_I = IH // 128         # 4 k-tiles over I-half
NTOK = 512               # full token tile (matmul moving dim)
TAIL = 256               # token padding granularity (f32r full rate needs >=256)
f32 = mybir.dt.float32
f32r = mybir.dt.float32r
ACT_FN = mybir.ActivationFunctionType.Silu  # test_sim overrides (CoreSim lacks Silu)


def _ensure_ntff_hook():
    """If the environment requests tracing (BASS_TRACE) but lacks the
    antenv.axon_hooks module, inject it so profiling works instead of being
    silently skipped."""
    try:
        from antenv.axon_hooks import get_axon_ntff_profile_hook  # noqa: F401
        return
    except ImportError:
        pass
    try:
        import sys, types
        import antenv
        from trn_agent_boot.trn_boot import _ntff_profile_via_ctypes
        mod = types.ModuleType("antenv.axon_hooks")
        mod._hook = _ntff_profile_via_ctypes("/opt/axon/libaxon_pjrt.so")
        mod.set_axon_ntff_profile_hook = lambda h: setattr(mod, "_hook", h)
        mod.get_axon_ntff_profile_hook = lambda: mod._hook
        sys.modules["antenv.axon_hooks"] = mod
        antenv.axon_hooks = mod
    except Exception:
        pass


_BUILD_CACHE = {}
LAST_RESULT = None


def build(r_tokens, tslice):
    """Build the SPMD program: R gathered tokens per core (multiple of TAIL)
    and a router slice of `tslice` tokens per core."""
    key = (r_tokens, tslice)
    if key in _BUILD_CACHE:
        return _BUILD_CACHE[key]
    assert r_tokens % TAIL == 0 and tslice % NTOK == 0
    ns_r = tslice // NTOK
    widths = [NTOK] * (r_tokens // NTOK)
    if r_tokens % NTOK:
        widths.append(TAIL)
    offs = np.cumsum([0] + widths)[:-1].tolist()
    R = r_tokens

    nc = bacc.Bacc("TRN2", target_bir_lowering=False, debug=False, num_devices=8)
    xtb = nc.dram_tensor("xtb", [H, R], f32r, kind="ExternalInput").ap()
    cwb = nc.dram_tensor("cwb", [128, R], f32, kind="ExternalInput").ap()
    w1tb = nc.dram_tensor("w1tb", [2, KT_H, 128, IH], f32r, kind="ExternalInput").ap()
    w3tb = nc.dram_tensor("w3tb", [2, KT_H, 128, IH], f32r, kind="ExternalInput").ap()
    w2tb = nc.dram_tensor("w2tb", [2, KT_I, 128, H], f32r, kind="ExternalInput").ap()
    xrb = nc.dram_tensor("xrb", [ns_r, KT_H, 128, NTOK], f32r, kind="ExternalInput").ap()
    gwtb = nc.dram_tensor("gwtb", [KT_H, 128, E], f32r, kind="ExternalInput").ap()
    yb = nc.dram_tensor("yb", [2, H, R], f32, kind="ExternalOutput").ap()
    logb = nc.dram_tensor("logb", [E, tslice], f32, kind="ExternalOutput").ap()

    xtb_k = xtb.rearrange("(kt p) r -> kt p r", p=128)

    with tile.TileContext(nc) as tc:
        with (
            tc.tile_pool(name="wp", bufs=1) as wp,
            tc.tile_pool(name="xp", bufs=2) as xp,
            tc.tile_pool(name="pp", bufs=2) as pp,
            tc.tile_pool(name="cp", bufs=2) as cp,
            tc.tile_pool(name="yp", bufs=4) as yp,
            tc.tile_pool(name="rp", bufs=2) as rp,
            tc.tile_pool(name="gups", bufs=4, space="PSUM") as gups,
            tc.tile_pool(name="yps", bufs=3, space="PSUM") as yps,
            tc.tile_pool(name="lps", bufs=1, space="PSUM") as lps,
        ):
            def ffn_half(h):
                w1t, w3t, w2t = [], [], []
                for ki in range(KT_H):
                    t1 = wp.tile([128, IH], f32r, tag=f"w1_{ki}", name=f"w1_{h}_{ki}")
                    nc.sync.dma_start(out=t1[:], in_=w1tb[h, ki])
                    w1t.append(t1)
                    t3 = wp.tile([128, IH], f32r, tag=f"w3_{ki}", name=f"w3_{h}_{ki}")
                    nc.sync.dma_start(out=t3[:], in_=w3tb[h, ki])
                    w3t.append(t3)
                for ki in range(KT_I):
                    t2 = wp.tile([128, H], f32r, tag=f"w2_{ki}", name=f"w2_{h}_{ki}")
                    nc.sync.dma_start(out=t2[:], in_=w2tb[h, ki])
                    w2t.append(t2)

                for ti, (off, w) in enumerate(zip(offs, widths)):
                    if h == 0 and ti < len(router_slices):
                        router_slices[ti]()
                    xt = xp.tile([128, KT_H * w], f32r, tag="x", name=f"x_{h}_{ti}")
                    for ki in range(KT_H):
                        nc.sync.dma_start(
                            out=xt[:, ki * w:(ki + 1) * w],
                            in_=xtb_k[ki][:, off:off + w],
                        )
                    cw = cp.tile([128, w], f32, tag="cw", bufs=1, name=f"cw_{h}_{ti}")
                    nc.sync.dma_start(out=cw[:], in_=cwb[:, off:off + w])

                    pt = pp.tile([128, ISUB * w], f32r, tag="p", name=f"p_{h}_{ti}")
                    for isub in range(ISUB):
                        gp = gups.tile([128, w], f32, tag="gu", name=f"g_{h}_{ti}_{isub}")
                        up = gups.tile([128, w], f32, tag="gu", name=f"u_{h}_{ti}_{isub}")
                        msl = slice(isub * 128, (isub + 1) * 128)
                        for ki in range(KT_H):
                            nc.tensor.matmul(
                                gp[:],
                                w1t[ki][:, msl],
                                xt[:, ki * w:(ki + 1) * w],
                                start=(ki == 0),
                                stop=(ki == KT_H - 1),
                            )
                        for ki in range(KT_H):
                            nc.tensor.matmul(
                                up[:],
                                w3t[ki][:, msl],
                                xt[:, ki * w:(ki + 1) * w],
                                start=(ki == 0),
                                stop=(ki == KT_H - 1),
                            )
                        psl = slice(isub * w, (isub + 1) * w)
                        sg = yp.tile([128, w], f32, tag="sg", bufs=1, name=f"sg_{h}_{ti}_{isub}")
                        nc.scalar.activation(sg[:], gp[:], ACT_FN)
                        # P = silu(G) * (U * cw); cw folded here (per-token scale)
                        nc.vector.tensor_mul(pt[:, psl], up[:], cw[:])
                        nc.vector.tensor_mul(pt[:, psl], pt[:, psl], sg[:])
                    for m in range(KT_H):
                        yps_t = yps.tile([128, w], f32, tag="y", name=f"y_{h}_{ti}_{m}")
                        for ki in range(KT_I):
                            nc.tensor.matmul(
                                yps_t[:],
                                w2t[ki][:, m * 128:(m + 1) * 128],
                                pt[:, ki * w:(ki + 1) * w],
                                start=(ki == 0),
                                stop=(ki == KT_I - 1),
                            )
                        ysb = yp.tile([128, w], f32, tag="ysb", bufs=3, name=f"ysb_{h}_{ti}_{m}")
                        nc.vector.tensor_copy(ysb[:], yps_t[:])
                        nc.sync.dma_start(
                            out=yb[h][m * 128:(m + 1) * 128, off:off + w], in_=ysb[:]
                        )

            gwt_box = []

            def router_slice(s):
                def emit():
                    if not gwt_box:
                        g = rp.tile([128, KT_H * E], f32r, tag="gwt", bufs=1, name="gwt")
                        for ki in range(KT_H):
                            nc.sync.dma_start(
                                out=g[:, ki * E:(ki + 1) * E], in_=gwtb[ki]
                            )
                        gwt_box.append(g)
                    gwt = gwt_box[0]
                    lp = lps.tile([E, NTOK], f32, tag="lp", name=f"lp_{s}")
                    for ch in range(4):
                        xr = xp.tile([128, 4 * NTOK], f32r, tag="xr", bufs=2,
                                     name=f"xr_{s}_{ch}")
                        for kj in range(4):
                            nc.sync.dma_start(
                                out=xr[:, kj * NTOK:(kj + 1) * NTOK],
                                in_=xrb[s, ch * 4 + kj],
                            )
                        for kj in range(4):
                            ki = ch * 4 + kj
                            nc.tensor.matmul(
                                lp[:],
                                gwt[:, ki * E:(ki + 1) * E],
                                xr[:, kj * NTOK:(kj + 1) * NTOK],
                                start=(ki == 0),
                                stop=(ki == KT_H - 1),
                            )
                    lsb = rp.tile([E, NTOK], f32, tag="lsb", bufs=1, name=f"lsb_{s}")
                    nc.vector.tensor_copy(lsb[:], lp[:])
                    nc.sync.dma_start(out=logb[:, s * NTOK:(s + 1) * NTOK], in_=lsb[:])
                return emit

            router_slices = [router_slice(s) for s in range(ns_r)]
            ffn_half(0)
            ffn_half(1)

    nc.compile()
    _BUILD_CACHE[key] = nc
    return nc


def kernel(hidden_states, gate_w, e_bias, w1, w2, w3):
    _ensure_ntff_hook()
    B, S, Hd = hidden_states.shape
    assert Hd == H
    x = np.ascontiguousarray(hidden_states.reshape(-1, H).astype(np.float32))
    T = x.shape[0]
    tslice = T // 8

    # ---- host routing (float64 for stable top-k decisions) ----
    logits64 = x.astype(np.float64) @ gate_w.T.astype(np.float64)
    scores = 1.0 / (1.0 + np.exp(-logits64))
    biased = scores + e_bias.astype(np.float64)
    sel = np.argsort(-biased, axis=1, kind="stable")[:, :TOP_K]       # [T, 2]
    rw = np.take_along_axis(scores, sel, axis=1)                       # [T, 2]
    rw = rw / np.maximum(rw.sum(-1, keepdims=True), 1e-12)
    rw = rw.astype(np.float32)

    idx = []
    wts = []
    for e in range(E):
        mask = sel == e                                                # [T, 2]
        ide = np.where(mask.any(1))[0]
        pos = mask[ide].argmax(1)
        idx.append(ide)
        wts.append(rw[ide, pos])

    nmax = max(1, max(len(i) for i in idx))
    R = ((nmax + TAIL - 1) // TAIL) * TAIL

    nc = build(R, tslice)

    # ---- per-core inputs ----
    w1T = np.ascontiguousarray(np.transpose(w1, (0, 2, 1)).astype(np.float32))  # [E, H, I]
    w3T = np.ascontiguousarray(np.transpose(w3, (0, 2, 1)).astype(np.float32))  # [E, H, I]
    w2T = np.ascontiguousarray(np.transpose(w2, (0, 2, 1)).astype(np.float32))  # [E, I, H]
    gwtb = np.ascontiguousarray(gate_w.T.astype(np.float32).reshape(KT_H, 128, E))

    in_maps = []
    for c in range(E):
        n_c = len(idx[c])
        xg = np.zeros((R, H), np.float32)
        xg[:n_c] = x[idx[c]]
        xtb = np.ascontiguousarray(xg.T)                               # [H, R]
        cwf = np.zeros((R,), np.float32)
        cwf[:n_c] = wts[c]
        cwb = np.ascontiguousarray(np.broadcast_to(cwf, (128, R)))
        w1tb = np.ascontiguousarray(
            w1T[c].reshape(KT_H, 128, 2, IH).transpose(2, 0, 1, 3)
        )  # [2, KT_H, 128, IH]
        w3tb = np.ascontiguousarray(
            w3T[c].reshape(KT_H, 128, 2, IH).transpose(2, 0, 1, 3)
        )
        w2tb = np.ascontiguousarray(w2T[c].reshape(2, KT_I, 128, H))
        xsl = x[c * tslice:(c + 1) * tslice]                           # [tslice, H]
        xrb = np.ascontiguousarray(
            xsl.reshape(tslice // NTOK, NTOK, H).transpose(0, 2, 1)
            .reshape(tslice // NTOK, KT_H, 128, NTOK)
        )
        in_maps.append(
            {
                "xtb": xtb,
                "cwb": cwb,
                "w1tb": w1tb,
                "w3tb": w3tb,
                "w2tb": w2tb,
                "xrb": xrb,
                "gwtb": gwtb,
            }
        )

    res = run_bass_kernel_spmd(nc, in_maps, list(range(8)))
    global LAST_RESULT
    LAST_RESULT = res

    # ---- combine on host ----
    out = np.zeros((T, H), np.float32)
    logits_out = np.empty((T, E), np.float32)
    for c in range(E):
        r = res.results[c]
        y = (r["yb"][0] + r["yb"][1]).T                                # [R, H]
        out[idx[c]] += y[: len(idx[c])]
        logits_out[c * tslice:(c + 1) * tslice] = r["logb"].T
    return out.reshape(B, S, H), logits_out


# revision 12
# speedup vs baseline: 1.0829x; 1.0405x over previous
"""MiniMax-M2 sparse MoE block on 8 Trainium2 NeuronCores.

Strategy: expert-parallel. Core c owns expert c's weights. The host computes
the routing (top-2 of 8, sigmoid scores + bias for selection) in float64,
gathers each expert's tokens, and ships them pre-transposed; each core runs
the gated FFN (silu(x@w1.T) * (x@w3.T)) @ w2.T over its gathered tokens in
float32r (full-rate fp32 matmul mode) and also computes the router logits for
its 1/8 slice of tokens. The host scatter-adds the weighted expert outputs.

Shapes (fixed by the problem): B=4, S=4096, H=2048, I=1024, E=8, top_k=2.

The intermediate dim I is processed in two halves so that one half's weights
(w1/w3 halves [H,512] + w2 half [512,H], fp32) stay resident in SBUF; each
half emits a partial Y (summed on the host). The router block is emitted
between the two halves so its matmuls run on a warm PE and overlap the
half-1 weight reload.
"""
import numpy as np

import concourse.bass as bass
import concourse.mybir as mybir
from concourse import bacc
import concourse.tile as tile
from concourse.bass_utils import run_bass_kernel_spmd

H = 2048
I = 1024
E = 8
TOP_K = 2
KT_H = H // 128          # 16 k-tiles over H
IH = I // 2              # 512: half of intermediate dim
ISUB = IH // 128         # 4 psum subtiles per half
KT_I = IH // 128         # 4 k-tiles over I-half
NTOK = 512               # full token tile (matmul moving dim)
TAIL = 256               # token padding granularity (f32r full rate needs >=256)
f32 = mybir.dt.float32
f32r = mybir.dt.float32r
ACT_FN = mybir.ActivationFunctionType.Silu  # test_sim overrides (CoreSim lacks Silu)


def _ensure_ntff_hook():
    """If the environment requests tracing (BASS_TRACE) but lacks the
    antenv.axon_hooks module, inject it so profiling works instead of being
    silently skipped."""
    try:
        from antenv.axon_hooks import get_axon_ntff_profile_hook  # noqa: F401
        return
    except ImportError:
        pass
    try:
        import sys, types
        import antenv
        from trn_agent_boot.trn_boot import _ntff_profile_via_ctypes
        mod = types.ModuleType("antenv.axon_hooks")
        mod._hook = _ntff_profile_via_ctypes("/opt/axon/libaxon_pjrt.so")
        mod.set_axon_ntff_profile_hook = lambda h: setattr(mod, "_hook", h)
        mod.get_axon_ntff_profile_hook = lambda: mod._hook
        sys.modules["antenv.axon_hooks"] = mod
        antenv.axon_hooks = mod
    except Exception:
        pass


_BUILD_CACHE = {}
LAST_RESULT = None


def build(r_tokens, tslice):
    """Build the SPMD program: R gathered tokens per core (multiple of TAIL)
    and a router slice of `tslice` tokens per core."""
    key = (r_tokens, tslice)
    if key in _BUILD_CACHE:
        return _BUILD_CACHE[key]
    assert r_tokens % TAIL == 0 and tslice % NTOK == 0
    ns_r = tslice // NTOK
    widths = [NTOK] * (r_tokens // NTOK)
    if r_tokens % NTOK:
        widths.append(TAIL)
    offs = np.cumsum([0] + widths)[:-1].tolist()
    R = r_tokens

    nc = bacc.Bacc("TRN2", target_bir_lowering=False, debug=False, num_devices=8)
    xtb = nc.dram_tensor("xtb", [H, R], f32r, kind="ExternalInput").ap()
    cwb = nc.dram_tensor("cwb", [128, R], f32, kind="ExternalInput").ap()
    w1tb = nc.dram_tensor("w1tb", [2, KT_H, 128, IH], f32r, kind="ExternalInput").ap()
    w3tb = nc.dram_tensor("w3tb", [2, KT_H, 128, IH], f32r, kind="ExternalInput").ap()
    w2tb = nc.dram_tensor("w2tb", [2, KT_I, 128, H], f32r, kind="ExternalInput").ap()
    xrb = nc.dram_tensor("xrb", [ns_r, KT_H, 128, NTOK], f32r, kind="ExternalInput").ap()
    gwtb = nc.dram_tensor("gwtb", [KT_H, 128, E], f32r, kind="ExternalInput").ap()
    yb = nc.dram_tensor("yb", [2, H, R], f32, kind="ExternalOutput").ap()
    logb = nc.dram_tensor("logb", [E, tslice], f32, kind="ExternalOutput").ap()

    xtb_k = xtb.rearrange("(kt p) r -> kt p r", p=128)

    with tile.TileContext(nc) as tc:
        with (
            tc.tile_pool(name="wp", bufs=1) as wp,
            tc.tile_pool(name="xp", bufs=2) as xp,
            tc.tile_pool(name="pp", bufs=2) as pp,
            tc.tile_pool(name="cp", bufs=2) as cp,
            tc.tile_pool(name="yp", bufs=4) as yp,
            tc.tile_pool(name="rp", bufs=2) as rp,
            tc.tile_pool(name="gups", bufs=4, space="PSUM") as gups,
            tc.tile_pool(name="yps", bufs=3, space="PSUM") as yps,
            tc.tile_pool(name="lps", bufs=1, space="PSUM") as lps,
        ):
            def load_tile_inputs(h, ti, off, w):
                xt = xp.tile([128, KT_H * w], f32r, tag="x", name=f"x_{h}_{ti}")
                for ki in range(KT_H):
                    nc.sync.dma_start(
                        out=xt[:, ki * w:(ki + 1) * w],
                        in_=xtb_k[ki][:, off:off + w],
                    )
                cw = cp.tile([128, w], f32, tag="cw", bufs=1, name=f"cw_{h}_{ti}")
                nc.sync.dma_start(out=cw[:], in_=cwb[:, off:off + w])
                return xt, cw

            def ffn_half(h):
                # tile-0 inputs (and the first router slice) are queued BEFORE
                # the 12.6MB weight block so PE work starts within a few us.
                if h == 0 and router_slices:
                    router_slices[0]()
                tile0 = load_tile_inputs(h, 0, offs[0], widths[0])

                w1t, w3t, w2t = [], [], []
                for ki in range(KT_H):
                    t1 = wp.tile([128, IH], f32r, tag=f"w1_{ki}", name=f"w1_{h}_{ki}")
                    nc.sync.dma_start(out=t1[:], in_=w1tb[h, ki])
                    w1t.append(t1)
                for ki in range(KT_H):
                    t3 = wp.tile([128, IH], f32r, tag=f"w3_{ki}", name=f"w3_{h}_{ki}")
                    nc.sync.dma_start(out=t3[:], in_=w3tb[h, ki])
                    w3t.append(t3)
                for ki in range(KT_I):
                    t2 = wp.tile([128, H], f32r, tag=f"w2_{ki}", name=f"w2_{h}_{ki}")
                    nc.sync.dma_start(out=t2[:], in_=w2tb[h, ki])
                    w2t.append(t2)

                for ti, (off, w) in enumerate(zip(offs, widths)):
                    if h == 0 and 0 < ti < len(router_slices):
                        router_slices[ti]()
                    if ti == 0:
                        xt, cw = tile0
                    else:
                        xt, cw = load_tile_inputs(h, ti, off, w)

                    pt = pp.tile([128, ISUB * w], f32r, tag="p", name=f"p_{h}_{ti}")
                    for isub in range(ISUB):
                        gp = gups.tile([128, w], f32, tag="gu", name=f"g_{h}_{ti}_{isub}")
                        up = gups.tile([128, w], f32, tag="gu", name=f"u_{h}_{ti}_{isub}")
                        msl = slice(isub * 128, (isub + 1) * 128)
                        for ki in range(KT_H):
                            nc.tensor.matmul(
                                gp[:],
                                w1t[ki][:, msl],
                                xt[:, ki * w:(ki + 1) * w],
                                start=(ki == 0),
                                stop=(ki == KT_H - 1),
                            )
                        for ki in range(KT_H):
                            nc.tensor.matmul(
                                up[:],
                                w3t[ki][:, msl],
                                xt[:, ki * w:(ki + 1) * w],
                                start=(ki == 0),
                                stop=(ki == KT_H - 1),
                            )
                        psl = slice(isub * w, (isub + 1) * w)
                        sg = yp.tile([128, w], f32, tag="sg", bufs=1, name=f"sg_{h}_{ti}_{isub}")
                        nc.scalar.activation(sg[:], gp[:], ACT_FN)
                        # P = silu(G) * (U * cw); cw folded here (per-token scale)
                        nc.vector.tensor_mul(pt[:, psl], up[:], cw[:])
                        nc.vector.tensor_mul(pt[:, psl], pt[:, psl], sg[:])
                    for m in range(KT_H):
                        yps_t = yps.tile([128, w], f32, tag="y", name=f"y_{h}_{ti}_{m}")
                        for ki in range(KT_I):
                            nc.tensor.matmul(
                                yps_t[:],
                                w2t[ki][:, m * 128:(m + 1) * 128],
                                pt[:, ki * w:(ki + 1) * w],
                                start=(ki == 0),
                                stop=(ki == KT_I - 1),
                            )
                        ysb = yp.tile([128, w], f32, tag="ysb", bufs=3, name=f"ysb_{h}_{ti}_{m}")
                        nc.vector.tensor_copy(ysb[:], yps_t[:])
                        nc.sync.dma_start(
                            out=yb[h][m * 128:(m + 1) * 128, off:off + w], in_=ysb[:]
                        )

            gwt_box = []

            def router_slice(s):
                def emit():
                    if not gwt_box:
                        g = rp.tile([128, KT_H * E], f32r, tag="gwt", bufs=1, name="gwt")
                        for ki in range(KT_H):
                            nc.sync.dma_start(
                                out=g[:, ki * E:(ki + 1) * E], in_=gwtb[ki]
                            )
                        gwt_box.append(g)
                    gwt = gwt_box[0]
                    lp = lps.tile([E, NTOK], f32, tag="lp", name=f"lp_{s}")
                    for ch in range(4):
                        xr = xp.tile([128, 4 * NTOK], f32r, tag="xr", bufs=2,
                                     name=f"xr_{s}_{ch}")
                        for kj in range(4):
                            nc.sync.dma_start(
                                out=xr[:, kj * NTOK:(kj + 1) * NTOK],
                                in_=xrb[s, ch * 4 + kj],
                            )
                        for kj in range(4):
                            ki = ch * 4 + kj
                            nc.tensor.matmul(
                                lp[:],
                                gwt[:, ki * E:(ki + 1) * E],
                                xr[:, kj * NTOK:(kj + 1) * NTOK],
                                start=(ki == 0),
                                stop=(ki == KT_H - 1),
                            )
                    lsb = rp.tile([E, NTOK], f32, tag="lsb", bufs=1, name=f"lsb_{s}")
                    nc.vector.tensor_copy(lsb[:], lp[:])
                    nc.sync.dma_start(out=logb[:, s * NTOK:(s + 1) * NTOK], in_=lsb[:])
                return emit

            router_slices = [router_slice(s) for s in range(ns_r)]
            ffn_half(0)
            ffn_half(1)

    nc.compile()
    _BUILD_CACHE[key] = nc
    return nc


def kernel(hidden_states, gate_w, e_bias, w1, w2, w3):
    _ensure_ntff_hook()
    B, S, Hd = hidden_states.shape
    assert Hd == H
    x = np.ascontiguousarray(hidden_states.reshape(-1, H).astype(np.float32))
    T = x.shape[0]
    tslice = T // 8

    # ---- host routing (float64 for stable top-k decisions) ----
    logits64 = x.astype(np.float64) @ gate_w.T.astype(np.float64)
    scores = 1.0 / (1.0 + np.exp(-logits64))
    biased = scores + e_bias.astype(np.float64)
    sel = np.argsort(-biased, axis=1, kind="stable")[:, :TOP_K]       # [T, 2]
    rw = np.take_along_axis(scores, sel, axis=1)                       # [T, 2]
    rw = rw / np.maximum(rw.sum(-1, keepdims=True), 1e-12)
    rw = rw.astype(np.float32)

    idx = []
    wts = []
    for e in range(E):
        mask = sel == e                                                # [T, 2]
        ide = np.where(mask.any(1))[0]
        pos = mask[ide].argmax(1)
        idx.append(ide)
        wts.append(rw[ide, pos])

    nmax = max(1, max(len(i) for i in idx))
    R = ((nmax + TAIL - 1) // TAIL) * TAIL

    nc = build(R, tslice)

    # ---- per-core inputs ----
    w1T = np.ascontiguousarray(np.transpose(w1, (0, 2, 1)).astype(np.float32))  # [E, H, I]
    w3T = np.ascontiguousarray(np.transpose(w3, (0, 2, 1)).astype(np.float32))  # [E, H, I]
    w2T = np.ascontiguousarray(np.transpose(w2, (0, 2, 1)).astype(np.float32))  # [E, I, H]
    gwtb = np.ascontiguousarray(gate_w.T.astype(np.float32).reshape(KT_H, 128, E))

    in_maps = []
    for c in range(E):
        n_c = len(idx[c])
        xg = np.zeros((R, H), np.float32)
        xg[:n_c] = x[idx[c]]
        xtb = np.ascontiguousarray(xg.T)                               # [H, R]
        cwf = np.zeros((R,), np.float32)
        cwf[:n_c] = wts[c]
        cwb = np.ascontiguousarray(np.broadcast_to(cwf, (128, R)))
        w1tb = np.ascontiguousarray(
            w1T[c].reshape(KT_H, 128, 2, IH).transpose(2, 0, 1, 3)
        )  # [2, KT_H, 128, IH]
        w3tb = np.ascontiguousarray(
            w3T[c].reshape(KT_H, 128, 2, IH).transpose(2, 0, 1, 3)
        )
        w2tb = np.ascontiguousarray(w2T[c].reshape(2, KT_I, 128, H))
        xsl = x[c * tslice:(c + 1) * tslice]                           # [tslice, H]
        xrb = np.ascontiguousarray(
            xsl.reshape(tslice // NTOK, NTOK, H).transpose(0, 2, 1)
            .reshape(tslice // NTOK, KT_H, 128, NTOK)
        )
        in_maps.append(
            {
                "xtb": xtb,
                "cwb": cwb,
                "w1tb": w1tb,
                "w3tb": w3tb,
                "w2tb": w2tb,
                "xrb": xrb,
                "gwtb": gwtb,
            }
        )

    res = run_bass_kernel_spmd(nc, in_maps, list(range(8)))
    global LAST_RESULT
    LAST_RESULT = res

    # ---- combine on host ----
    out = np.zeros((T, H), np.float32)
    logits_out = np.empty((T, E), np.float32)
    for c in range(E):
        r = res.results[c]
        y = (r["yb"][0] + r["yb"][1]).T                                # [R, H]
        out[idx[c]] += y[: len(idx[c])]
        logits_out[c * tslice:(c + 1) * tslice] = r["logb"].T
    return out.reshape(B, S, H), logits_out
